# revision 41
# baseline (speedup 1.0000x reference)
"""Trainium2 Bass kernel for nn_MAR_52209622450490 (OctFormer sparse attention).

Sharding: depth2batch gather applied host-side while sharding - each core gets
a contiguous 2048-token slice of the window-ordered token stream (multiple of
the 512-token super-window), so all 4 blocks plus both loss heads are fully
core-local; each core emits 4 partial sums combined on host.

Perf design (cost-model driven):
 - All dense matmuls run fp8(e4m3) DoubleRow (0.5 cyc/row, K=256/instr).
   Weights x4 host-side, activations /4 folded into LN rstd (or rz for the
   attention output), so PSUM results are exact-scale f32.
 - Transposed operands (HT/OT) are stored as fp8 channel-PAIRS viewed as
   uint16: one XBAR DMA transpose moves 2 channels/partition; the DoubleRow
   k-tile dim then indexes pair parity. Weight rows are pair-reshaped host-side.
 - Scores: Q/K evacuated fp8 as [h*16+dlo partition, dhalf, tok]; per-head
   16-row stationary with DoubleRow over dhalf halves scores PE cost.
 - Softmax exp: scores land in 2-bank PSUM tiles ([128,2,512]), ONE ACT exp
   per 4 head-chunks ([128,1024]) - ACT per-instr overhead ~185ns dominates
   otherwise. Second matmul per bank uses start=False (bank's pending-zero
   from the first start covers it).
 - LN rstd via batched ACT ln->exp (natural_log_exp table) instead of Sqrt:
   keeps ONE activation table across LN+softmax+heads; only gelu swaps
   tables (2 loads/block instead of ~10).
 - AV uses DoubleRow over the two key-chunks; V carries a ones-column so
   each PSUM tile holds its softmax normalizer Z.
 - DVE keeps LN stats/applies + residual adds (critical path); PSUM
   evacuation copies and the VQ-head group-sum reduces go to the idle
   Pool/GPSIMD engine.
"""
import numpy as np
import ml_dtypes

import concourse.tile as tile
from concourse import bacc, mybir
from concourse.bass_utils import run_bass_kernel_spmd
from concourse.masks import make_identity

N_SPLIT = 4096
N_VQ = 12288
N = N_SPLIT + N_VQ
C = 256
H = 8
DH = 32
L = 4
P = 256
DIL = 2
HID = 4 * C
VQ_G = 4
VQ_SIZE = 256
NCORES = 8
T = N // NCORES            # 2048 tokens per core
TC = T // 128              # 16 row-tiles per core
NWIN = T // P              # 8 windows per core
EPS = 1e-5
SCALE = DH ** -0.5
S = 4.0                    # fp8 operand balancing scale

F32 = mybir.dt.float32
BF16 = mybir.dt.bfloat16
F8 = mybir.dt.float8e4
U16 = mybir.dt.uint16
BF = ml_dtypes.bfloat16
E4M3 = ml_dtypes.float8_e4m3
DR = mybir.MatmulPerfMode.DoubleRow

_CACHE = {}


def _sin_pos_emb(n, c):
    pos = np.arange(n, dtype=np.float32)[:, None]
    half = c // 2
    freqs = np.exp(-np.log(10000.0) * np.arange(half, dtype=np.float32) / half)
    ang = pos * freqs
    return np.concatenate([np.sin(ang), np.cos(ang)], axis=-1).astype(np.float32)


def _st(beg, cnt, step):
    return slice(beg, beg + (cnt - 1) * step + 1, step)


from contextlib import ExitStack as _ES

PRIO_OFF = 700


def build_nc(flags, n_blocks=L):
    nc = bacc.Bacc(None, target_bir_lowering=False)

    d_emb = nc.declare_dram_parameter("emb", [T, C], F32, isOutput=False)
    d_zqt = nc.declare_dram_parameter("zqt", [DH, T], BF16, isOutput=False)
    d_vqpw = nc.declare_dram_parameter("vqpw", [DH, C], BF16, isOutput=False)
    d_wqkv = nc.declare_dram_parameter("wqkv", [L, 128, 2, 2 * C], F8, isOutput=False)
    d_wattn = nc.declare_dram_parameter("wattn", [L, 128, 2, C], F8, isOutput=False)
    d_wfc1 = nc.declare_dram_parameter("wfc1", [L, 128, 2, HID], F8, isOutput=False)
    d_wfc2 = nc.declare_dram_parameter("wfc2", [L, 128, 8, C], F8, isOutput=False)
    d_wvq = nc.declare_dram_parameter("wvq", [128, 2, VQ_G * VQ_SIZE], BF16, isOutput=False)
    d_wspl = nc.declare_dram_parameter("wspl", [128, 2, 2], BF16, isOutput=False)
    d_wvv = nc.declare_dram_parameter("wvv", [L, 128, 2, C], BF16, isOutput=False)
    d_bqk = nc.declare_dram_parameter("bqk", [L, 128, 4], F32, isOutput=False)
    d_bv = nc.declare_dram_parameter("bv", [L, 2 * C], F32, isOutput=False)
    d_battn = nc.declare_dram_parameter("battn", [L, 2 * C], F32, isOutput=False)
    d_bfc1 = nc.declare_dram_parameter("bfc1", [L, 128, 8], F32, isOutput=False)
    d_bfc2 = nc.declare_dram_parameter("bfc2", [L, 2 * C], F32, isOutput=False)
    d_bspl = nc.declare_dram_parameter("bspl", [32], F32, isOutput=False)
    d_ebq = nc.declare_dram_parameter("ebq", [VQ_G * VQ_SIZE], F32, isOutput=False)
    d_wsel = nc.declare_dram_parameter("wsel", [T, C], F32, isOutput=False)
    d_bsel = nc.declare_dram_parameter("bsel", [T], F32, isOutput=False)
    d_msc = nc.declare_dram_parameter("msc", [T], F32, isOutput=False)
    d_mvc = nc.declare_dram_parameter("mvc", [T], F32, isOutput=False)
    d_stc = nc.declare_dram_parameter("stc", [T], F32, isOutput=False)
    d_out = nc.declare_dram_parameter("out", [128, 4], F32, isOutput=True)

    LN_EXP = mybir.ActivationFunctionType.Ln
    EXP = mybir.ActivationFunctionType.Exp
    GELU = mybir.ActivationFunctionType.Gelu_apprx_tanh
    IDENT = mybir.ActivationFunctionType.Identity
    ADD = mybir.AluOpType.add
    SUB = mybir.AluOpType.subtract
    MULT = mybir.AluOpType.mult

    with tile.TileContext(nc) as tc:
        with (
            tc.tile_pool(name="big", bufs=1) as big,
            tc.tile_pool(name="wpool", bufs=2) as wp,
            tc.tile_pool(name="small", bufs=1) as sm,
            tc.tile_pool(name="trans", bufs=8) as tr,
            tc.tile_pool(name="lnp", bufs=2) as lnp,
            tc.tile_pool(name="hbp", bufs=6) as hbp,
            tc.tile_pool(name="ebpool", bufs=5) as ebp,
            tc.tile_pool(name="evpool", bufs=3) as evp,
            tc.tile_pool(name="gtpool", bufs=2) as gtp,
            tc.tile_pool(name="ps2p", bufs=2, space="PSUM") as ps2p,
            tc.tile_pool(name="mmp", bufs=2, space="PSUM") as mmp,
            tc.tile_pool(name="avp", bufs=1, space="PSUM") as avp,
            tc.tile_pool(name="ptrp", bufs=1, space="PSUM") as ptrp,
        ):
            XB = big.tile([128, TC, C], F32, tag="XB")
            HT2 = big.tile([128, T], BF16, tag="HT2")         # fp8 pair-interleave
            # partition (h%4)*32+d, dim1 = head group h//4
            QT = big.tile([128, 2, T], BF16, tag="QT")
            KT = big.tile([128, 2, T], BF16, tag="KT")
            VB = big.tile([128, 2 * NWIN, H, DH + 1], F8, tag="VB")
            HTV = big.tile([128, 2, T], BF16, tag="HTV")
            OROW = big.tile([128, TC, C], F8, tag="OROW")
            OT2 = big.tile([128, T], BF16, tag="OT2")

            def pair8(u16_tile):
                return u16_tile[:].bitcast(F8).rearrange("p (t two) -> p two t", two=2)

            HT8 = pair8(HT2)
            OT8 = pair8(OT2)

            ident = sm.tile([128, 128], BF16, tag="ident")
            make_identity(nc, ident[:])
            epsT = sm.tile([128, 1], F32, tag="eps")
            nc.vector.memset(epsT[:], EPS)
            nLnS = sm.tile([128, 1], F32, tag="nLnS")
            nc.vector.memset(nLnS[:], -float(np.log(S)))
            zqt = sm.tile([DH, T], BF16, tag="zqt")
            vqpw = sm.tile([DH, C], BF16, tag="vqpw")

            nc.vector.memset(VB[:, :, :, DH], 1.0)

            # ---------------- embed (chunked so LN1 starts early) ----------------
            demb = d_emb.rearrange("(t p) c -> p t c", p=128)
            nc.sync.dma_start(zqt[:, 0:512], d_zqt[:, 0:512])
            nc.sync.dma_start(vqpw[:], d_vqpw[:])
            for q in range(4):
                nc.sync.dma_start(XB[:, 4 * q:4 * q + 4, :],
                                  demb[:, 4 * q:4 * q + 4, :])
                if q > 0:
                    nc.sync.dma_start(zqt[:, q * 512:(q + 1) * 512],
                                      d_zqt[:, q * 512:(q + 1) * 512])
            for u in range(TC // 2):
                ps = mmp.tile([128, 512], F32, tag="mm")
                for half in range(2):
                    t = 2 * u + half
                    nc.tensor.matmul(ps[:, half * C:(half + 1) * C],
                                     zqt[:, t * 128:(t + 1) * 128], vqpw[:],
                                     start=(half == 0), stop=True,
                                     skip_group_check=True)
                xpair = XB[:, 2 * u:2 * u + 2, :].rearrange("p t c -> p (t c)")
                nc.vector.tensor_tensor(xpair, xpair, ps[:], ADD)

            def prio(cond):
                s = _ES()
                if cond:
                    s.enter_context(tc.high_priority(offset=PRIO_OFF))
                return s

            def ln_pass(dstT2, after_apply=None, backdate_first=False,
                        want_nmr=False, make_htv=False):
                """LayerNorm XB -> fp8/S pair-interleaved into dstT2 (u16).
                Returns (MV, RSTD, NMR)."""
                MV = lnp.tile([128, TC, 2], F32, tag="MV")
                RSTD = lnp.tile([128, TC], F32, tag="RSTD")
                if want_nmr:
                    NMR = lnp.tile([128, TC], F32, tag="NMR")
                else:
                    NMR = None
                for grp in range(4):
                    with prio(backdate_first and grp == 0):
                        for tt in range(4):
                            t = grp * 4 + tt
                            st6 = tr.tile([128, 6], F32, tag="bn6")
                            nc.vector.bn_stats(st6[:], XB[:, t, :])
                            nc.vector.bn_aggr(MV[:, t, :], st6[:])
                        gs = slice(grp * 4, grp * 4 + 4)
                        lnv = tr.tile([128, 4], F32, tag="lnv")
                        nc.scalar.activation(lnv[:], MV[:, gs, 1], LN_EXP,
                                             bias=epsT[:])
                        nc.scalar.activation(RSTD[:, gs], lnv[:], EXP,
                                             scale=-0.5, bias=nLnS[:])
                        if want_nmr:
                            nc.vector.scalar_tensor_tensor(
                                NMR[:, gs], MV[:, gs, 0], -1.0, RSTD[:, gs],
                                MULT, MULT)
                        for tt in range(4):
                            t = grp * 4 + tt
                            hbf = hbp.tile([128, C], F8, tag="hbf")
                            nc.vector.tensor_scalar(hbf[:], XB[:, t, :],
                                                    MV[:, t, 0:1],
                                                    RSTD[:, t:t + 1],
                                                    SUB, MULT)
                            nc.sync.dma_start_transpose(
                                dstT2[:, t * 128:(t + 1) * 128],
                                hbf[:].bitcast(BF16))
                            if make_htv:
                                hbv = hbp.tile([128, C], BF16, tag="hbv")
                                nc.vector.tensor_scalar(hbv[:], XB[:, t, :],
                                                        MV[:, t, 0:1],
                                                        RSTD[:, t:t + 1],
                                                        SUB, MULT)
                                pvt = ptrp.tile([128, C], BF16, tag="pvt")
                                for cc in range(2):
                                    nc.tensor.transpose(
                                        pvt[:, cc * 128:(cc + 1) * 128],
                                        hbv[:, cc * 128:(cc + 1) * 128],
                                        ident[:])
                                nc.vector.tensor_copy(
                                    HTV[:, :, t * 128:(t + 1) * 128],
                                    pvt[:].rearrange("p (a b) -> p a b", a=2))
                            if after_apply is not None:
                                after_apply(t)
                return MV, RSTD, NMR

            # ---------------- transformer blocks ----------------
            for l in range(n_blocks):
                wqkv = wp.tile([128, 2, 2 * C], F8, tag="wqkv")
                nc.sync.dma_start(wqkv[:], d_wqkv[l])
                wattn = wp.tile([128, 2, C], F8, tag="wattn")
                wvv = wp.tile([128, 2, C], BF16, tag="wvv")
                nc.sync.dma_start(wvv[:], d_wvv[l])
                nc.sync.dma_start(wattn[:], d_wattn[l])
                wfc1 = wp.tile([128, 2, HID], F8, tag="wfc1")
                nc.sync.dma_start(wfc1[:], d_wfc1[l])
                wfc2 = wp.tile([128, 8, C], F8, tag="wfc2")
                nc.sync.dma_start(wfc2[:], d_wfc2[l])
                bqk = vbb = abb = f1b = f2b = None
                if flags["bqkv"]:
                    bqk = wp.tile([128, 4], F32, tag="bqk")
                    nc.sync.dma_start(bqk[:], d_bqk[l])
                if flags["bqkv_v"]:
                    vbb = wp.tile([128, 2 * C], F32, tag="vbb")
                    nc.sync.dma_start(vbb[:], d_bv[l].to_broadcast([128, 2 * C]))
                if flags["battn"]:
                    abb = wp.tile([128, 2 * C], F32, tag="abb")
                    nc.sync.dma_start(abb[:], d_battn[l].to_broadcast([128, 2 * C]))
                if flags["bfc1"]:
                    f1b = wp.tile([128, 8], F32, tag="f1b")
                    nc.sync.dma_start(f1b[:], d_bfc1[l])
                if flags["bfc2"]:
                    f2b = wp.tile([128, 2 * C], F32, tag="f2b")
                    nc.sync.dma_start(f2b[:], d_bfc2[l].to_broadcast([128, 2 * C]))

                dil = DIL if (l % 2 == 1) else 1

                ln_pass(HT2, backdate_first=(l > 0), make_htv=True)

                # Interleaved per 512-token group q: QKV(q) -> V -> scores/AV
                # for windows 2q, 2q+1 (keeps the ps2 pool FIFO from
                # serializing whole phases against each other).
                def qkv_group(q, fast):
                    tok = slice(q * 512, (q + 1) * 512)
                    for qk in range(2):           # 0: Q, 1: K
                        dstT = QT if qk == 0 else KT
                        ps = ps2p.tile([128, 2, 512], F32, tag="ps2")
                        for gg in range(2):
                            g = qk * 2 + gg
                            nc.tensor.matmul(
                                ps[:, gg, :],
                                wqkv[:, :, g * 128:(g + 1) * 128],
                                HT8[:, :, tok],
                                start=True, stop=True, perf_mode=DR)
                        if flags["bqkv"]:
                            for gg in range(2):
                                bi = qk * 2 + gg
                                nc.scalar.activation(
                                    dstT[:, gg, tok], ps[:, gg, :], IDENT,
                                    bias=bqk[:, bi:bi + 1])
                        else:
                            nc.vector.tensor_copy(dstT[:, :, tok], ps[:])

                def v_window(wlin):
                    sw, r = divmod(wlin, dil)
                    start = sw * P * dil + r
                    ps = mmp.tile([128, 512], F32, tag="mm")
                    for qc in range(2):
                        tok = _st(start + qc * 128 * dil, 128, dil)
                        for cc in range(2):
                            nc.tensor.matmul(ps[:, qc * C:(qc + 1) * C],
                                             HTV[:, cc, tok],
                                             wvv[:, cc, :],
                                             start=(qc == 0 and cc == 0),
                                             stop=(cc == 1),
                                             skip_group_check=True)
                    vdst = VB[:, wlin * 2:wlin * 2 + 2, :, 0:DH]
                    psv = ps[:].rearrange("p (k h d) -> p k h d", k=2, h=H)
                    if flags["bqkv_v"]:
                        nc.vector.tensor_tensor(
                            vdst, psv,
                            vbb[:].rearrange("p (k h d) -> p k h d",
                                             k=2, h=H), ADD)
                    else:
                        nc.vector.tensor_copy(vdst, psv)

                def attn_window(wlin):
                    sw, r = divmod(wlin, dil)
                    start = sw * P * dil + r
                    alltok = _st(start, P, dil)
                    EB = ebp.tile([128, 2, H, P], F8, tag="EB")
                    for rr in range(4):           # head pairs
                        pss = ps2p.tile([128, 2, 512], F32, tag="ps2")
                        for ho in range(2):
                            hh = 2 * rr + ho
                            gg = hh // 4
                            hp = slice((hh % 4) * 32, (hh % 4) * 32 + 32)
                            for kc in range(2):
                                ktok = _st(start + kc * 128 * dil, 128, dil)
                                nc.tensor.matmul(
                                    pss[:, kc, ho * 256:(ho + 1) * 256],
                                    KT[hp, gg, ktok],
                                    QT[hp, gg, alltok],
                                    start=(ho == 0), stop=True,
                                    skip_group_check=True,
                                    tile_position=((hh % 4) * 32, 0))
                        nc.scalar.activation(
                            EB[:, :, 2 * rr:2 * rr + 2, :], pss[:],
                            EXP, scale=SCALE / 4.0)

                    for wc in range(2):
                        pso = avp.tile([128, H, 64], F32, tag="av")
                        for hh in range(H):
                            nc.tensor.matmul(
                                pso[:, hh, 0:DH + 1],
                                EB[:, :, hh, wc * 128:(wc + 1) * 128],
                                VB[:, wlin * 2:wlin * 2 + 2, hh, :],
                                start=(hh == 0), stop=True, perf_mode=DR,
                                skip_group_check=True)
                        rz = tr.tile([128, H], F32, tag="rz")
                        nc.vector.reciprocal(rz[:], pso[:, :, DH])
                        wi = wlin * 2 + wc
                        nc.vector.scalar_tensor_tensor(
                            OROW[:, wi, :].rearrange("p (h d) -> p h d", h=H),
                            pso[:, :, 0:DH], 1.0 / S,
                            rz[:, :, None].to_broadcast([128, H, DH]),
                            MULT, MULT)
                        dtok = _st(start + wc * 128 * dil, 128, dil)
                        if dil == 1:
                            nc.sync.dma_start_transpose(
                                OT2[:, start + wc * 128:start + wc * 128 + 128],
                                OROW[:, wi, :].bitcast(BF16))
                        else:
                            pst = ptrp.tile([128, C], BF16, tag="pvt")
                            nc.tensor.transpose(pst[:, 0:128],
                                                OROW[:, wi, :].bitcast(BF16),
                                                ident[:])
                            nc.vector.tensor_copy(OT2[:, dtok], pst[:, 0:128])

                with prio(l > 0):
                    qkv_group(0, fast=True)
                    v_window(0)
                    v_window(1)
                for q in range(4):
                    with prio(l > 0 and q == 0):
                        attn_window(2 * q)
                        if q < 3:
                            qkv_group(q + 1, fast=False)
                            v_window(2 * q + 2)
                            v_window(2 * q + 3)
                        attn_window(2 * q + 1)

                # attn proj + residual + LN2 + fc1 + gelu + fc2, pipelined
                def attnproj_pair(u):
                    ps = mmp.tile([128, 512], F32, tag="mm")
                    for half in range(2):
                        t = 2 * u + half
                        nc.tensor.matmul(ps[:, half * C:(half + 1) * C],
                                         OT8[:, :, t * 128:(t + 1) * 128],
                                         wattn[:], start=(half == 0),
                                         stop=True, perf_mode=DR,
                                         skip_group_check=True)
                    xpair = XB[:, 2 * u:2 * u + 2, :].rearrange("p t c -> p (t c)")
                    nc.vector.tensor_tensor(xpair, xpair, ps[:], ADD)
                    if flags["battn"]:
                        nc.vector.tensor_tensor(xpair, xpair, abb[:], ADD)

                for u in range(TC // 2):
                    attnproj_pair(u)

                ln_pass(HT2)

                for quart in range(4):
                    with prio(quart == 0):
                        tok = slice(quart * 512, (quart + 1) * 512)
                        GTq = gtp.tile([128, HID // 128, 512], F8, tag="GTq")
                        for mu in range(4):
                            ps = ps2p.tile([128, 2, 512], F32, tag="ps2")
                            for mo in range(2):
                                m = 2 * mu + mo
                                nc.tensor.matmul(
                                    ps[:, mo, :],
                                    wfc1[:, :, m * 128:(m + 1) * 128],
                                    HT8[:, :, tok],
                                    start=True, stop=True, perf_mode=DR)
                            gdst = GTq[:, 2 * mu:2 * mu + 2, :]
                            if flags["bfc1"]:
                                for mo in range(2):
                                    nc.scalar.activation(
                                        gdst[:, mo, :], ps[:, mo, :], GELU,
                                        bias=f1b[:, 2 * mu + mo:2 * mu + mo + 1])
                            else:
                                nc.scalar.activation(gdst, ps[:], GELU)
                        for uu in range(2):
                            ps = mmp.tile([128, 512], F32, tag="mm")
                            for to in range(2):
                                tth = 2 * uu + to
                                for hc in range(4):
                                    nc.tensor.matmul(
                                        ps[:, to * C:(to + 1) * C],
                                        GTq[:, 2 * hc:2 * hc + 2,
                                            tth * 128:(tth + 1) * 128],
                                        wfc2[:, 2 * hc:2 * hc + 2, :],
                                        start=(to == 0 and hc == 0),
                                        stop=(hc == 3), perf_mode=DR,
                                        skip_group_check=True)
                            t0 = 4 * quart + 2 * uu
                            xpair = XB[:, t0:t0 + 2, :].rearrange("p t c -> p (t c)")
                            nc.vector.scalar_tensor_tensor(
                                xpair, ps[:], 1.0 / S, xpair, MULT, ADD)
                            if flags["bfc2"]:
                                nc.vector.tensor_tensor(xpair, xpair, f2b[:], ADD)

            # ---------------- heads ----------------
            wvq = sm.tile([128, 2, VQ_G * VQ_SIZE], BF16, tag="wvq")
            nc.sync.dma_start(wvq[:], d_wvq[:])
            wspl = sm.tile([128, 2, 2], BF16, tag="wspl")
            nc.sync.dma_start(wspl[:], d_wspl[:])
            wselB = sm.tile([128, TC, C], F32, tag="wsel")
            nc.sync.dma_start(wselB[:], d_wsel.rearrange("(t p) c -> p t c", p=128))
            MSC = sm.tile([128, TC], F32, tag="msc")
            nc.sync.dma_start(MSC[:], d_msc.rearrange("(t p) -> p t", p=128))
            MVC = sm.tile([128, TC], F32, tag="mvc")
            nc.sync.dma_start(MVC[:], d_mvc.rearrange("(t p) -> p t", p=128))
            STC = sm.tile([128, TC], F32, tag="stc")
            nc.sync.dma_start(STC[:], d_stc.rearrange("(t p) -> p t", p=128))
            if flags["bsel"]:
                BSL = sm.tile([128, TC], F32, tag="bsel")
                nc.sync.dma_start(BSL[:], d_bsel.rearrange("(t p) -> p t", p=128))
            if flags["ebq"]:
                EBQ = sm.tile([128, VQ_G * VQ_SIZE], F32, tag="ebq")
                nc.sync.dma_start(EBQ[:], d_ebq.to_broadcast([128, VQ_G * VQ_SIZE]))
            if flags["bspl"]:
                BSP = sm.tile([128, 32], F32, tag="bspl")
                nc.sync.dma_start(BSP[:], d_bspl.to_broadcast([128, 32]))

            SLB = sm.tile([128, TC, 2], F32, tag="SLB")
            GSL = sm.tile([128, TC, VQ_G], F32, tag="GSL")
            TSB = sm.tile([128, TC], F32, tag="TSB")
            SCR = sm.tile([128, C], F32, tag="SCR")

            # final LN -> XNT2 (reuse HT2); TSB via fused affine-mul-reduce
            MVf, RSTDf, NMRf = ln_pass(HT2, want_nmr=True, make_htv=True)
            for t in range(TC):
                nc.vector.affine_mul_reduce(
                    SCR[:], TSB[:, t:t + 1], XB[:, t, :], wselB[:, t, :],
                    RSTDf[:, t:t + 1], NMRf[:, t:t + 1])

            # split head: all 16 chunks into one PSUM bank
            psl = mmp.tile([128, 512], F32, tag="mm")
            for t in range(TC):
                for cc in range(2):
                    nc.tensor.matmul(psl[:, t * 2:t * 2 + 2],
                                     HTV[:, cc, t * 128:(t + 1) * 128],
                                     wspl[:, cc, :],
                                     start=(t == 0 and cc == 0),
                                     stop=(cc == 1),
                                     skip_group_check=True)
            if flags["bspl"]:
                nc.vector.tensor_tensor(
                    SLB[:].rearrange("p t g -> p (t g)"), psl[:, 0:32],
                    BSP[:], ADD)
            else:
                nc.vector.tensor_copy(SLB[:].rearrange("p t g -> p (t g)"),
                                      psl[:, 0:32])

            # vq head
            for t in range(TC):
                ps = ps2p.tile([128, 2, 512], F32, tag="ps2")
                for nk in range(2):
                    for cc in range(2):
                        nc.tensor.matmul(ps[:, nk, :],
                                         HTV[:, cc, t * 128:(t + 1) * 128],
                                         wvq[:, cc, nk * 512:(nk + 1) * 512],
                                         start=(cc == 0), stop=(cc == 1))
                EV = evp.tile([128, VQ_G * VQ_SIZE], BF16, tag="EV")
                nc.scalar.activation(EV[:], ps[:], EXP)
                if flags["ebq"]:
                    nc.vector.tensor_tensor(EV[:], EV[:], EBQ[:], MULT)
                nc.vector.tensor_reduce(
                    GSL[:, t, :],
                    EV[:].rearrange("p (g v) -> p g v", g=VQ_G),
                    mybir.AxisListType.X, ADD)

            # ce_v = 0.25*(sum_g ln GSL_g) - 0.25*(TSB [+bsel])
            LGS = sm.tile([128, TC, VQ_G], F32, tag="LGS")
            nc.scalar.activation(
                LGS[:].rearrange("p t g -> p (t g)"),
                GSL[:].rearrange("p t g -> p (t g)"), LN_EXP)
            CEV = sm.tile([128, TC], F32, tag="CEV")
            nc.vector.tensor_reduce(CEV[:], LGS[:],
                                    mybir.AxisListType.X, ADD)
            nc.vector.tensor_sub(CEV[:], CEV[:], TSB[:])
            if flags["bsel"]:
                nc.vector.tensor_sub(CEV[:], CEV[:], BSL[:])
            nc.vector.tensor_scalar_mul(CEV[:], CEV[:], 0.25)

            # ce_s = ln(exp(sl0)+exp(sl1)) - (sl0 + st*(sl1-sl0))
            ES = sm.tile([128, TC, 2], F32, tag="ES")
            nc.scalar.activation(ES[:].rearrange("p t g -> p (t g)"),
                                 SLB[:].rearrange("p t g -> p (t g)"), EXP)
            CES = sm.tile([128, TC], F32, tag="CES")
            nc.vector.tensor_reduce(CES[:], ES[:],
                                    mybir.AxisListType.X, ADD)
            nc.scalar.activation(CES[:], CES[:], LN_EXP)
            DD = sm.tile([128, TC], F32, tag="DD")
            nc.vector.tensor_sub(DD[:], SLB[:, :, 1], SLB[:, :, 0])
            nc.vector.tensor_tensor(DD[:], DD[:], STC[:], MULT)
            nc.vector.tensor_add(DD[:], DD[:], SLB[:, :, 0])
            nc.vector.tensor_sub(CES[:], CES[:], DD[:])

            R4 = sm.tile([128, 4], F32, tag="R4")
            W1 = sm.tile([128, TC], F32, tag="W1")
            nc.vector.tensor_tensor(W1[:], CES[:], MSC[:], MULT)
            W2 = sm.tile([128, TC], F32, tag="W2")
            nc.vector.tensor_tensor(W2[:], CEV[:], MVC[:], MULT)
            for i, srcbuf in enumerate([W1, MSC, W2, MVC]):
                rtc = tr.tile([128, 1], F32, tag="rtc")
                nc.vector.tensor_reduce(rtc[:], srcbuf[:],
                                        mybir.AxisListType.X, ADD)
                nc.vector.tensor_copy(R4[:, i:i + 1], rtc[:])

            nc.sync.dma_start(d_out[:], R4[:])

    nc.compile()
    return nc


def prepare_inputs(inputs):
    """Host-side: fold LN into weights, apply d2b permutation, fp8 packing,
    shard."""
    split = np.asarray(inputs["split"]).astype(np.int64)
    zq = np.asarray(inputs["zq"], dtype=np.float32)
    targets_vq = np.asarray(inputs["targets_vq"]).astype(np.int64)
    category = np.asarray(inputs["category"]).astype(np.int64)
    batch_id = np.asarray(inputs["batch_id"]).astype(np.int64)
    mask = np.asarray(inputs["mask"]).astype(bool)
    d2b = np.asarray(inputs["d2b"]).astype(np.int64)
    g = lambda k: np.asarray(inputs[k], dtype=np.float32)
    split_emb, class_emb = g("split_emb"), g("class_emb")
    vq_proj_w, vq_proj_b = g("vq_proj_w"), g("vq_proj_b")
    ln1_s, ln1_b = g("ln1_s"), g("ln1_b")
    qkv_w, qkv_b = g("qkv_w"), g("qkv_b")
    attn_w, attn_b = g("attn_w"), g("attn_b")
    ln2_s, ln2_b = g("ln2_s"), g("ln2_b")
    fc1_w, fc1_b = g("fc1_w"), g("fc1_b")
    fc2_w, fc2_b = g("fc2_w"), g("fc2_b")
    lnx_s, lnx_b = g("lnx_s"), g("lnx_b")
    split_w, split_b = g("split_w"), g("split_b")
    vq_w, vq_b = g("vq_w"), g("vq_b")

    # LN folds
    qkv_w_eff = ln1_s[:, :, None] * qkv_w                       # [L,C,3C]
    qkv_b_eff = np.einsum("lc,lcn->ln", ln1_b, qkv_w) + qkv_b   # [L,3C]
    fc1_w_eff = ln2_s[:, :, None] * fc1_w
    fc1_b_eff = np.einsum("lc,lcn->ln", ln2_b, fc1_w) + fc1_b
    vq_w_eff = lnx_s[:, None] * vq_w
    vq_b_eff = lnx_b @ vq_w + vq_b
    spl_w_eff = lnx_s[:, None] * split_w
    spl_b_eff = lnx_b @ split_w + split_b

    # natural col order [Q|K] fp8 x2S; V separate bf16 cc-major xS
    def pack_qkv(w):        # [C, 3C] -> [128, 2, 2C]
        qw, kw = w[:, 0:C], w[:, C:2 * C]
        cols = np.concatenate([qw * (2 * S), kw * (2 * S)], axis=1)
        return cols.reshape(128, 2, 2 * C)

    wqkv2 = np.stack([pack_qkv(qkv_w_eff[l]) for l in range(L)])
    wattn2 = (attn_w * S).reshape(L, 128, 2, C)
    wfc12 = (fc1_w_eff * S).reshape(L, 128, 2, HID)
    wfc22 = (fc2_w * S).reshape(L, 8, 128, C).transpose(0, 2, 1, 3)
    wvq2 = (vq_w_eff * S).reshape(2, 128, VQ_G * VQ_SIZE).transpose(1, 0, 2)
    wspl2 = (spl_w_eff * S).reshape(2, 128, 2).transpose(1, 0, 2)
    wvv2 = (qkv_w_eff[:, :, 2 * C:3 * C] * S).reshape(L, 2, 128, C).transpose(0, 2, 1, 3)

    # biases (normally all zero)
    bqk2 = np.zeros((L, 128, 4), np.float32)
    for l in range(L):
        for g in range(4):
            qk, gg = divmod(g, 2)
            bqk2[l, :, g] = qkv_b_eff[l, qk * C + gg * 128:
                                      qk * C + gg * 128 + 128] * 2
    bv2 = np.tile(qkv_b_eff[:, 2 * C:3 * C], (1, 2))
    battn2 = np.tile(attn_b, (1, 2))
    bfc12 = fc1_b_eff.reshape(L, 8, 128).transpose(0, 2, 1)
    bfc22 = np.tile(fc2_b, (1, 2))
    bspl2 = np.tile(spl_b_eff, 16)

    # token embedding pieces, depth order
    cond_rows = class_emb[category[batch_id]]                   # [N,C]
    base_depth = np.empty((N, C), np.float32)
    base_depth[:N_SPLIT] = split_emb[split]
    base_depth[N_SPLIT:] = vq_proj_b[None, :]
    base_depth[mask] = cond_rows[mask]
    zq_depth = np.zeros((N, DH), np.float32)
    zq_depth[N_SPLIT:] = zq
    zq_depth[mask] = 0.0

    ms_depth = np.zeros(N, np.float32)
    ms_depth[:N_SPLIT] = mask[:N_SPLIT]
    mv_depth = np.zeros(N, np.float32)
    mv_depth[N_SPLIT:] = mask[N_SPLIT:]
    st_depth = np.zeros(N, np.float32)
    st_depth[:N_SPLIT] = split
    wsel_depth = np.zeros((N, C), np.float32)
    cols = targets_vq + np.arange(VQ_G)[None, :] * VQ_SIZE      # [N_VQ,4]
    wsel_depth[N_SPLIT:] = vq_w_eff.T[cols].sum(axis=1)         # [N_VQ,C]
    bsel_depth = np.zeros(N, np.float32)
    bsel_depth[N_SPLIT:] = vq_b_eff[cols].sum(axis=1)

    # window order + positional embedding
    pe = _sin_pos_emb(N, C)
    emb_w = base_depth[d2b] + pe
    zq_w = zq_depth[d2b]
    ms_w, mv_w, st_w = ms_depth[d2b], mv_depth[d2b], st_depth[d2b]
    wsel_w, bsel_w = wsel_depth[d2b] * S, bsel_depth[d2b]

    flags = {
        "bqkv": bool(np.any(qkv_b_eff[:, :2 * C])),
        "bqkv_v": bool(np.any(qkv_b_eff[:, 2 * C:])),
        "battn": bool(np.any(attn_b)),
        "bfc1": bool(np.any(fc1_b_eff)),
        "bfc2": bool(np.any(fc2_b)),
        "bspl": bool(np.any(spl_b_eff)),
        "bsel": bool(np.any(bsel_w)),
        "ebq": bool(np.any(vq_b_eff)),
    }

    shared = {
        "vqpw": vq_proj_w.astype(BF),
        "wqkv": wqkv2.astype(E4M3),
        "wattn": wattn2.astype(E4M3),
        "wfc1": wfc12.astype(E4M3),
        "wfc2": wfc22.astype(E4M3),
        "wvq": wvq2.astype(BF),
        "wspl": wspl2.astype(BF),
        "wvv": wvv2.astype(BF),
        "bqk": bqk2.astype(np.float32),
        "bv": bv2.astype(np.float32),
        "battn": battn2.astype(np.float32),
        "bfc1": bfc12.astype(np.float32),
        "bfc2": bfc22.astype(np.float32),
        "bspl": bspl2.astype(np.float32),
        "ebq": np.exp(vq_b_eff).astype(np.float32),
    }
    in_maps = []
    for c in range(NCORES):
        s = slice(c * T, (c + 1) * T)
        m = dict(shared)
        m["emb"] = np.ascontiguousarray(emb_w[s])
        m["zqt"] = np.ascontiguousarray(zq_w[s].T).astype(BF)
        m["wsel"] = np.ascontiguousarray(wsel_w[s])
        m["bsel"] = np.ascontiguousarray(bsel_w[s])
        m["msc"] = np.ascontiguousarray(ms_w[s])
        m["mvc"] = np.ascontiguousarray(mv_w[s])
        m["stc"] = np.ascontiguousarray(st_w[s])
        in_maps.append(m)
    return in_maps, flags


def kernel(**inputs) -> np.ndarray:
    in_maps, flags = prepare_inputs(inputs)
    key = tuple(sorted(flags.items()))
    if key not in _CACHE:
        _CACHE[key] = build_nc(flags)
    nc = _CACHE[key]
    res = run_bass_kernel_spmd(nc, in_maps, core_ids=list(range(NCORES)))
    parts = np.stack([res.results[c]["out"].sum(axis=0) for c in range(NCORES)])
    s = parts.sum(axis=0)
    split_loss = s[0] / max(s[1], 1.0)
    vq_loss = s[2] / max(s[3], 1.0)
    return np.stack([split_loss, vq_loss]).astype(np.float32)


# revision 42
# speedup vs baseline: 1.0063x; 1.0063x over previous
"""Trainium2 Bass kernel for nn_MAR_52209622450490 (OctFormer sparse attention).

Sharding: depth2batch gather applied host-side while sharding - each core gets
a contiguous 2048-token slice of the window-ordered token stream (multiple of
the 512-token super-window), so all 4 blocks plus both loss heads are fully
core-local; each core emits 4 partial sums combined on host.

Perf design (cost-model driven):
 - All dense matmuls run fp8(e4m3) DoubleRow (0.5 cyc/row, K=256/instr).
   Weights x4 host-side, activations /4 folded into LN rstd (or rz for the
   attention output), so PSUM results are exact-scale f32.
 - Transposed operands (HT/OT) are stored as fp8 channel-PAIRS viewed as
   uint16: one XBAR DMA transpose moves 2 channels/partition; the DoubleRow
   k-tile dim then indexes pair parity. Weight rows are pair-reshaped host-side.
 - Scores: Q/K evacuated fp8 as [h*16+dlo partition, dhalf, tok]; per-head
   16-row stationary with DoubleRow over dhalf halves scores PE cost.
 - Softmax exp: scores land in 2-bank PSUM tiles ([128,2,512]), ONE ACT exp
   per 4 head-chunks ([128,1024]) - ACT per-instr overhead ~185ns dominates
   otherwise. Second matmul per bank uses start=False (bank's pending-zero
   from the first start covers it).
 - LN rstd via batched ACT ln->exp (natural_log_exp table) instead of Sqrt:
   keeps ONE activation table across LN+softmax+heads; only gelu swaps
   tables (2 loads/block instead of ~10).
 - AV uses DoubleRow over the two key-chunks; V carries a ones-column so
   each PSUM tile holds its softmax normalizer Z.
 - DVE keeps LN stats/applies + residual adds (critical path); PSUM
   evacuation copies and the VQ-head group-sum reduces go to the idle
   Pool/GPSIMD engine.
"""
import numpy as np
import ml_dtypes

import concourse.tile as tile
from concourse import bacc, mybir
from concourse.bass_utils import run_bass_kernel_spmd
from concourse.masks import make_identity

N_SPLIT = 4096
N_VQ = 12288
N = N_SPLIT + N_VQ
C = 256
H = 8
DH = 32
L = 4
P = 256
DIL = 2
HID = 4 * C
VQ_G = 4
VQ_SIZE = 256
NCORES = 8
T = N // NCORES            # 2048 tokens per core
TC = T // 128              # 16 row-tiles per core
NWIN = T // P              # 8 windows per core
EPS = 1e-5
SCALE = DH ** -0.5
S = 4.0                    # fp8 operand balancing scale

F32 = mybir.dt.float32
BF16 = mybir.dt.bfloat16
F8 = mybir.dt.float8e4
U16 = mybir.dt.uint16
BF = ml_dtypes.bfloat16
E4M3 = ml_dtypes.float8_e4m3
DR = mybir.MatmulPerfMode.DoubleRow

_CACHE = {}


def _sin_pos_emb(n, c):
    pos = np.arange(n, dtype=np.float32)[:, None]
    half = c // 2
    freqs = np.exp(-np.log(10000.0) * np.arange(half, dtype=np.float32) / half)
    ang = pos * freqs
    return np.concatenate([np.sin(ang), np.cos(ang)], axis=-1).astype(np.float32)


def _st(beg, cnt, step):
    return slice(beg, beg + (cnt - 1) * step + 1, step)


from contextlib import ExitStack as _ES

PRIO_OFF = 700


def build_nc(flags, n_blocks=L):
    nc = bacc.Bacc(None, target_bir_lowering=False)

    d_emb = nc.declare_dram_parameter("emb", [T, C], F32, isOutput=False)
    d_zqt = nc.declare_dram_parameter("zqt", [DH, T], BF16, isOutput=False)
    d_vqpw = nc.declare_dram_parameter("vqpw", [DH, C], BF16, isOutput=False)
    d_wqkv = nc.declare_dram_parameter("wqkv", [L, 128, 2, 2 * C], F8, isOutput=False)
    d_wattn = nc.declare_dram_parameter("wattn", [L, 128, 2, C], BF16, isOutput=False)
    d_wfc1 = nc.declare_dram_parameter("wfc1", [L, 128, 2, HID], F8, isOutput=False)
    d_wfc2 = nc.declare_dram_parameter("wfc2", [L, 128, 8, C], F8, isOutput=False)
    d_wvq = nc.declare_dram_parameter("wvq", [128, 2, VQ_G * VQ_SIZE], BF16, isOutput=False)
    d_wspl = nc.declare_dram_parameter("wspl", [128, 2, 2], BF16, isOutput=False)
    d_wvv = nc.declare_dram_parameter("wvv", [L, 128, 2, C], BF16, isOutput=False)
    d_bqk = nc.declare_dram_parameter("bqk", [L, 128, 4], F32, isOutput=False)
    d_bv = nc.declare_dram_parameter("bv", [L, 2 * C], F32, isOutput=False)
    d_battn = nc.declare_dram_parameter("battn", [L, 2 * C], F32, isOutput=False)
    d_bfc1 = nc.declare_dram_parameter("bfc1", [L, 128, 8], F32, isOutput=False)
    d_bfc2 = nc.declare_dram_parameter("bfc2", [L, 2 * C], F32, isOutput=False)
    d_bspl = nc.declare_dram_parameter("bspl", [32], F32, isOutput=False)
    d_ebq = nc.declare_dram_parameter("ebq", [VQ_G * VQ_SIZE], F32, isOutput=False)
    d_wsel = nc.declare_dram_parameter("wsel", [T, C], F32, isOutput=False)
    d_bsel = nc.declare_dram_parameter("bsel", [T], F32, isOutput=False)
    d_msc = nc.declare_dram_parameter("msc", [T], F32, isOutput=False)
    d_mvc = nc.declare_dram_parameter("mvc", [T], F32, isOutput=False)
    d_stc = nc.declare_dram_parameter("stc", [T], F32, isOutput=False)
    d_out = nc.declare_dram_parameter("out", [128, 4], F32, isOutput=True)

    LN_EXP = mybir.ActivationFunctionType.Ln
    EXP = mybir.ActivationFunctionType.Exp
    GELU = mybir.ActivationFunctionType.Gelu_apprx_tanh
    IDENT = mybir.ActivationFunctionType.Identity
    ADD = mybir.AluOpType.add
    SUB = mybir.AluOpType.subtract
    MULT = mybir.AluOpType.mult

    with tile.TileContext(nc) as tc:
        with (
            tc.tile_pool(name="big", bufs=1) as big,
            tc.tile_pool(name="wpool", bufs=2) as wp,
            tc.tile_pool(name="small", bufs=1) as sm,
            tc.tile_pool(name="trans", bufs=8) as tr,
            tc.tile_pool(name="lnp", bufs=2) as lnp,
            tc.tile_pool(name="hbp", bufs=6) as hbp,
            tc.tile_pool(name="ebpool", bufs=5) as ebp,
            tc.tile_pool(name="evpool", bufs=3) as evp,
            tc.tile_pool(name="gtpool", bufs=2) as gtp,
            tc.tile_pool(name="ps2p", bufs=2, space="PSUM") as ps2p,
            tc.tile_pool(name="mmp", bufs=2, space="PSUM") as mmp,
            tc.tile_pool(name="avp", bufs=1, space="PSUM") as avp,
            tc.tile_pool(name="ptrp", bufs=1, space="PSUM") as ptrp,
        ):
            XB = big.tile([128, TC, C], F32, tag="XB")
            HT2 = big.tile([128, T], BF16, tag="HT2")         # fp8 pair-interleave
            # partition (h%4)*32+d, dim1 = head group h//4
            QT = big.tile([128, 2, T], BF16, tag="QT")
            KT = big.tile([128, 2, T], BF16, tag="KT")
            VB = big.tile([128, 2 * NWIN, H, DH + 1], F8, tag="VB")
            HTV = big.tile([128, 2, T], BF16, tag="HTV")
            OROW = big.tile([128, TC, C], BF16, tag="OROW")
            OTV = big.tile([128, 2, T], BF16, tag="OTV")

            def pair8(u16_tile):
                return u16_tile[:].bitcast(F8).rearrange("p (t two) -> p two t", two=2)

            HT8 = pair8(HT2)

            ident = sm.tile([128, 128], BF16, tag="ident")
            make_identity(nc, ident[:])
            epsT = sm.tile([128, 1], F32, tag="eps")
            nc.vector.memset(epsT[:], EPS)
            nLnS = sm.tile([128, 1], F32, tag="nLnS")
            nc.vector.memset(nLnS[:], -float(np.log(S)))
            zqt = sm.tile([DH, T], BF16, tag="zqt")
            vqpw = sm.tile([DH, C], BF16, tag="vqpw")

            nc.vector.memset(VB[:, :, :, DH], 1.0)

            # ---------------- embed (chunked so LN1 starts early) ----------------
            demb = d_emb.rearrange("(t p) c -> p t c", p=128)
            nc.sync.dma_start(zqt[:, 0:512], d_zqt[:, 0:512])
            nc.sync.dma_start(vqpw[:], d_vqpw[:])
            for q in range(4):
                nc.sync.dma_start(XB[:, 4 * q:4 * q + 4, :],
                                  demb[:, 4 * q:4 * q + 4, :])
                if q > 0:
                    nc.sync.dma_start(zqt[:, q * 512:(q + 1) * 512],
                                      d_zqt[:, q * 512:(q + 1) * 512])
            for u in range(TC // 2):
                ps = mmp.tile([128, 512], F32, tag="mm")
                for half in range(2):
                    t = 2 * u + half
                    nc.tensor.matmul(ps[:, half * C:(half + 1) * C],
                                     zqt[:, t * 128:(t + 1) * 128], vqpw[:],
                                     start=(half == 0), stop=True,
                                     skip_group_check=True)
                xpair = XB[:, 2 * u:2 * u + 2, :].rearrange("p t c -> p (t c)")
                nc.vector.tensor_tensor(xpair, xpair, ps[:], ADD)

            def prio(cond):
                s = _ES()
                if cond:
                    s.enter_context(tc.high_priority(offset=PRIO_OFF))
                return s

            def ln_pass(dstT2, after_apply=None, backdate_first=False,
                        want_nmr=False, make_htv=False):
                """LayerNorm XB -> fp8/S pair-interleaved into dstT2 (u16).
                Returns (MV, RSTD, NMR)."""
                MV = lnp.tile([128, TC, 2], F32, tag="MV")
                RSTD = lnp.tile([128, TC], F32, tag="RSTD")
                if want_nmr:
                    NMR = lnp.tile([128, TC], F32, tag="NMR")
                else:
                    NMR = None
                for grp in range(4):
                    with prio(backdate_first and grp == 0):
                        for tt in range(4):
                            t = grp * 4 + tt
                            st6 = tr.tile([128, 6], F32, tag="bn6")
                            nc.vector.bn_stats(st6[:], XB[:, t, :])
                            nc.vector.bn_aggr(MV[:, t, :], st6[:])
                        gs = slice(grp * 4, grp * 4 + 4)
                        lnv = tr.tile([128, 4], F32, tag="lnv")
                        nc.scalar.activation(lnv[:], MV[:, gs, 1], LN_EXP,
                                             bias=epsT[:])
                        nc.scalar.activation(RSTD[:, gs], lnv[:], EXP,
                                             scale=-0.5, bias=nLnS[:])
                        if want_nmr:
                            nc.vector.scalar_tensor_tensor(
                                NMR[:, gs], MV[:, gs, 0], -1.0, RSTD[:, gs],
                                MULT, MULT)
                        for tt in range(4):
                            t = grp * 4 + tt
                            hbf = hbp.tile([128, C], F8, tag="hbf")
                            nc.vector.tensor_scalar(hbf[:], XB[:, t, :],
                                                    MV[:, t, 0:1],
                                                    RSTD[:, t:t + 1],
                                                    SUB, MULT)
                            nc.sync.dma_start_transpose(
                                dstT2[:, t * 128:(t + 1) * 128],
                                hbf[:].bitcast(BF16))
                            if make_htv:
                                hbv = hbp.tile([128, C], BF16, tag="hbv")
                                nc.vector.tensor_scalar(hbv[:], XB[:, t, :],
                                                        MV[:, t, 0:1],
                                                        RSTD[:, t:t + 1],
                                                        SUB, MULT)
                                pvt = ptrp.tile([128, C], BF16, tag="pvt")
                                for cc in range(2):
                                    nc.tensor.transpose(
                                        pvt[:, cc * 128:(cc + 1) * 128],
                                        hbv[:, cc * 128:(cc + 1) * 128],
                                        ident[:])
                                nc.vector.tensor_copy(
                                    HTV[:, :, t * 128:(t + 1) * 128],
                                    pvt[:].rearrange("p (a b) -> p a b", a=2))
                            if after_apply is not None:
                                after_apply(t)
                return MV, RSTD, NMR

            # ---------------- transformer blocks ----------------
            for l in range(n_blocks):
                wqkv = wp.tile([128, 2, 2 * C], F8, tag="wqkv")
                nc.sync.dma_start(wqkv[:], d_wqkv[l])
                wattn = wp.tile([128, 2, C], BF16, tag="wattn")
                wvv = wp.tile([128, 2, C], BF16, tag="wvv")
                nc.sync.dma_start(wvv[:], d_wvv[l])
                nc.sync.dma_start(wattn[:], d_wattn[l])
                wfc1 = wp.tile([128, 2, HID], F8, tag="wfc1")
                nc.sync.dma_start(wfc1[:], d_wfc1[l])
                wfc2 = wp.tile([128, 8, C], F8, tag="wfc2")
                nc.sync.dma_start(wfc2[:], d_wfc2[l])
                bqk = vbb = abb = f1b = f2b = None
                if flags["bqkv"]:
                    bqk = wp.tile([128, 4], F32, tag="bqk")
                    nc.sync.dma_start(bqk[:], d_bqk[l])
                if flags["bqkv_v"]:
                    vbb = wp.tile([128, 2 * C], F32, tag="vbb")
                    nc.sync.dma_start(vbb[:], d_bv[l].to_broadcast([128, 2 * C]))
                if flags["battn"]:
                    abb = wp.tile([128, 2 * C], F32, tag="abb")
                    nc.sync.dma_start(abb[:], d_battn[l].to_broadcast([128, 2 * C]))
                if flags["bfc1"]:
                    f1b = wp.tile([128, 8], F32, tag="f1b")
                    nc.sync.dma_start(f1b[:], d_bfc1[l])
                if flags["bfc2"]:
                    f2b = wp.tile([128, 2 * C], F32, tag="f2b")
                    nc.sync.dma_start(f2b[:], d_bfc2[l].to_broadcast([128, 2 * C]))

                dil = DIL if (l % 2 == 1) else 1

                ln_pass(HT2, backdate_first=(l > 0), make_htv=True)

                # Interleaved per 512-token group q: QKV(q) -> V -> scores/AV
                # for windows 2q, 2q+1 (keeps the ps2 pool FIFO from
                # serializing whole phases against each other).
                def qkv_group(q, fast):
                    tok = slice(q * 512, (q + 1) * 512)
                    for qk in range(2):           # 0: Q, 1: K
                        dstT = QT if qk == 0 else KT
                        ps = ps2p.tile([128, 2, 512], F32, tag="ps2")
                        for gg in range(2):
                            g = qk * 2 + gg
                            nc.tensor.matmul(
                                ps[:, gg, :],
                                wqkv[:, :, g * 128:(g + 1) * 128],
                                HT8[:, :, tok],
                                start=True, stop=True, perf_mode=DR)
                        if flags["bqkv"]:
                            for gg in range(2):
                                bi = qk * 2 + gg
                                nc.scalar.activation(
                                    dstT[:, gg, tok], ps[:, gg, :], IDENT,
                                    bias=bqk[:, bi:bi + 1])
                        else:
                            nc.vector.tensor_copy(dstT[:, :, tok], ps[:])

                def v_window(wlin):
                    sw, r = divmod(wlin, dil)
                    start = sw * P * dil + r
                    ps = mmp.tile([128, 512], F32, tag="mm")
                    for qc in range(2):
                        tok = _st(start + qc * 128 * dil, 128, dil)
                        for cc in range(2):
                            nc.tensor.matmul(ps[:, qc * C:(qc + 1) * C],
                                             HTV[:, cc, tok],
                                             wvv[:, cc, :],
                                             start=(qc == 0 and cc == 0),
                                             stop=(cc == 1),
                                             skip_group_check=True)
                    vdst = VB[:, wlin * 2:wlin * 2 + 2, :, 0:DH]
                    psv = ps[:].rearrange("p (k h d) -> p k h d", k=2, h=H)
                    if flags["bqkv_v"]:
                        nc.vector.tensor_tensor(
                            vdst, psv,
                            vbb[:].rearrange("p (k h d) -> p k h d",
                                             k=2, h=H), ADD)
                    else:
                        nc.vector.tensor_copy(vdst, psv)

                def attn_window(wlin):
                    sw, r = divmod(wlin, dil)
                    start = sw * P * dil + r
                    alltok = _st(start, P, dil)
                    EB = ebp.tile([128, 2, H, P], F8, tag="EB")
                    for rr in range(4):           # head pairs
                        pss = ps2p.tile([128, 2, 512], F32, tag="ps2")
                        for ho in range(2):
                            hh = 2 * rr + ho
                            gg = hh // 4
                            hp = slice((hh % 4) * 32, (hh % 4) * 32 + 32)
                            for kc in range(2):
                                ktok = _st(start + kc * 128 * dil, 128, dil)
                                nc.tensor.matmul(
                                    pss[:, kc, ho * 256:(ho + 1) * 256],
                                    KT[hp, gg, ktok],
                                    QT[hp, gg, alltok],
                                    start=(ho == 0), stop=True,
                                    skip_group_check=True,
                                    tile_position=((hh % 4) * 32, 0))
                        nc.scalar.activation(
                            EB[:, :, 2 * rr:2 * rr + 2, :], pss[:],
                            EXP, scale=SCALE / 4.0)

                    for wc in range(2):
                        pso = avp.tile([128, H, 64], F32, tag="av")
                        for hh in range(H):
                            nc.tensor.matmul(
                                pso[:, hh, 0:DH + 1],
                                EB[:, :, hh, wc * 128:(wc + 1) * 128],
                                VB[:, wlin * 2:wlin * 2 + 2, hh, :],
                                start=(hh == 0), stop=True, perf_mode=DR,
                                skip_group_check=True)
                        rz = tr.tile([128, H], F32, tag="rz")
                        nc.vector.reciprocal(rz[:], pso[:, :, DH])
                        wi = wlin * 2 + wc
                        nc.vector.scalar_tensor_tensor(
                            OROW[:, wi, :].rearrange("p (h d) -> p h d", h=H),
                            pso[:, :, 0:DH], 1.0 / S,
                            rz[:, :, None].to_broadcast([128, H, DH]),
                            MULT, MULT)
                        dtok = _st(start + wc * 128 * dil, 128, dil)
                        pst = ptrp.tile([128, C], BF16, tag="pvt")
                        for cc in range(2):
                            nc.tensor.transpose(
                                pst[:, cc * 128:(cc + 1) * 128],
                                OROW[:, wi, cc * 128:(cc + 1) * 128],
                                ident[:])
                        nc.vector.tensor_copy(
                            OTV[:, :, dtok],
                            pst[:].rearrange("p (a b) -> p a b", a=2))

                for q in range(4):
                    with prio(l > 0 and q == 0):
                        attn_window(2 * q)
                        if q < 3:
                            qkv_group(q + 1, fast=False)
                            v_window(2 * q + 2)
                            v_window(2 * q + 3)
                        attn_window(2 * q + 1)

                # attn proj + residual + LN2 + fc1 + gelu + fc2, pipelined
                def attnproj_pair(u):
                    ps = mmp.tile([128, 512], F32, tag="mm")
                    for half in range(2):
                        t = 2 * u + half
                        for cc in range(2):
                            nc.tensor.matmul(ps[:, half * C:(half + 1) * C],
                                             OTV[:, cc, t * 128:(t + 1) * 128],
                                             wattn[:, cc, :],
                                             start=(half == 0 and cc == 0),
                                             stop=(cc == 1),
                                             skip_group_check=True)
                    xpair = XB[:, 2 * u:2 * u + 2, :].rearrange("p t c -> p (t c)")
                    nc.vector.tensor_tensor(xpair, xpair, ps[:], ADD)
                    if flags["battn"]:
                        nc.vector.tensor_tensor(xpair, xpair, abb[:], ADD)

                for u in range(TC // 2):
                    attnproj_pair(u)

                ln_pass(HT2)

                for quart in range(4):
                    with prio(quart == 0):
                        tok = slice(quart * 512, (quart + 1) * 512)
                        GTq = gtp.tile([128, HID // 128, 512], F8, tag="GTq")
                        for mu in range(4):
                            ps = ps2p.tile([128, 2, 512], F32, tag="ps2")
                            for mo in range(2):
                                m = 2 * mu + mo
                                nc.tensor.matmul(
                                    ps[:, mo, :],
                                    wfc1[:, :, m * 128:(m + 1) * 128],
                                    HT8[:, :, tok],
                                    start=True, stop=True, perf_mode=DR)
                            gdst = GTq[:, 2 * mu:2 * mu + 2, :]
                            if flags["bfc1"]:
                                for mo in range(2):
                                    nc.scalar.activation(
                                        gdst[:, mo, :], ps[:, mo, :], GELU,
                                        bias=f1b[:, 2 * mu + mo:2 * mu + mo + 1])
                            else:
                                nc.scalar.activation(gdst, ps[:], GELU)
                        for uu in range(2):
                            ps = mmp.tile([128, 512], F32, tag="mm")
                            for to in range(2):
                                tth = 2 * uu + to
                                for hc in range(4):
                                    nc.tensor.matmul(
                                        ps[:, to * C:(to + 1) * C],
                                        GTq[:, 2 * hc:2 * hc + 2,
                                            tth * 128:(tth + 1) * 128],
                                        wfc2[:, 2 * hc:2 * hc + 2, :],
                                        start=(to == 0 and hc == 0),
                                        stop=(hc == 3), perf_mode=DR,
                                        skip_group_check=True)
                            t0 = 4 * quart + 2 * uu
                            xpair = XB[:, t0:t0 + 2, :].rearrange("p t c -> p (t c)")
                            nc.vector.scalar_tensor_tensor(
                                xpair, ps[:], 1.0 / S, xpair, MULT, ADD)
                            if flags["bfc2"]:
                                nc.vector.tensor_tensor(xpair, xpair, f2b[:], ADD)

            # ---------------- heads ----------------
            wvq = sm.tile([128, 2, VQ_G * VQ_SIZE], BF16, tag="wvq")
            nc.sync.dma_start(wvq[:], d_wvq[:])
            wspl = sm.tile([128, 2, 2], BF16, tag="wspl")
            nc.sync.dma_start(wspl[:], d_wspl[:])
            wselB = sm.tile([128, TC, C], F32, tag="wsel")
            nc.sync.dma_start(wselB[:], d_wsel.rearrange("(t p) c -> p t c", p=128))
            MSC = sm.tile([128, TC], F32, tag="msc")
            nc.sync.dma_start(MSC[:], d_msc.rearrange("(t p) -> p t", p=128))
            MVC = sm.tile([128, TC], F32, tag="mvc")
            nc.sync.dma_start(MVC[:], d_mvc.rearrange("(t p) -> p t", p=128))
            STC = sm.tile([128, TC], F32, tag="stc")
            nc.sync.dma_start(STC[:], d_stc.rearrange("(t p) -> p t", p=128))
            if flags["bsel"]:
                BSL = sm.tile([128, TC], F32, tag="bsel")
                nc.sync.dma_start(BSL[:], d_bsel.rearrange("(t p) -> p t", p=128))
            if flags["ebq"]:
                EBQ = sm.tile([128, VQ_G * VQ_SIZE], F32, tag="ebq")
                nc.sync.dma_start(EBQ[:], d_ebq.to_broadcast([128, VQ_G * VQ_SIZE]))
            if flags["bspl"]:
                BSP = sm.tile([128, 32], F32, tag="bspl")
                nc.sync.dma_start(BSP[:], d_bspl.to_broadcast([128, 32]))

            SLB = sm.tile([128, TC, 2], F32, tag="SLB")
            GSL = sm.tile([128, TC, VQ_G], F32, tag="GSL")
            TSB = sm.tile([128, TC], F32, tag="TSB")
            SCR = sm.tile([128, C], F32, tag="SCR")

            # final LN -> XNT2 (reuse HT2); TSB via fused affine-mul-reduce
            MVf, RSTDf, NMRf = ln_pass(HT2, want_nmr=True, make_htv=True)
            for t in range(TC):
                nc.vector.affine_mul_reduce(
                    SCR[:], TSB[:, t:t + 1], XB[:, t, :], wselB[:, t, :],
                    RSTDf[:, t:t + 1], NMRf[:, t:t + 1])

            # split head: all 16 chunks into one PSUM bank
            psl = mmp.tile([128, 512], F32, tag="mm")
            for t in range(TC):
                for cc in range(2):
                    nc.tensor.matmul(psl[:, t * 2:t * 2 + 2],
                                     HTV[:, cc, t * 128:(t + 1) * 128],
                                     wspl[:, cc, :],
                                     start=(t == 0 and cc == 0),
                                     stop=(cc == 1),
                                     skip_group_check=True)
            if flags["bspl"]:
                nc.vector.tensor_tensor(
                    SLB[:].rearrange("p t g -> p (t g)"), psl[:, 0:32],
                    BSP[:], ADD)
            else:
                nc.vector.tensor_copy(SLB[:].rearrange("p t g -> p (t g)"),
                                      psl[:, 0:32])

            # vq head
            for t in range(TC):
                ps = ps2p.tile([128, 2, 512], F32, tag="ps2")
                for nk in range(2):
                    for cc in range(2):
                        nc.tensor.matmul(ps[:, nk, :],
                                         HTV[:, cc, t * 128:(t + 1) * 128],
                                         wvq[:, cc, nk * 512:(nk + 1) * 512],
                                         start=(cc == 0), stop=(cc == 1))
                EV = evp.tile([128, VQ_G * VQ_SIZE], BF16, tag="EV")
                nc.scalar.activation(EV[:], ps[:], EXP)
                if flags["ebq"]:
                    nc.vector.tensor_tensor(EV[:], EV[:], EBQ[:], MULT)
                nc.vector.tensor_reduce(
                    GSL[:, t, :],
                    EV[:].rearrange("p (g v) -> p g v", g=VQ_G),
                    mybir.AxisListType.X, ADD)

            # ce_v = 0.25*(sum_g ln GSL_g) - 0.25*(TSB [+bsel])
            LGS = sm.tile([128, TC, VQ_G], F32, tag="LGS")
            nc.scalar.activation(
                LGS[:].rearrange("p t g -> p (t g)"),
                GSL[:].rearrange("p t g -> p (t g)"), LN_EXP)
            CEV = sm.tile([128, TC], F32, tag="CEV")
            nc.vector.tensor_reduce(CEV[:], LGS[:],
                                    mybir.AxisListType.X, ADD)
            nc.vector.tensor_sub(CEV[:], CEV[:], TSB[:])
            if flags["bsel"]:
                nc.vector.tensor_sub(CEV[:], CEV[:], BSL[:])
            nc.vector.tensor_scalar_mul(CEV[:], CEV[:], 0.25)

            # ce_s = ln(exp(sl0)+exp(sl1)) - (sl0 + st*(sl1-sl0))
            ES = sm.tile([128, TC, 2], F32, tag="ES")
            nc.scalar.activation(ES[:].rearrange("p t g -> p (t g)"),
                                 SLB[:].rearrange("p t g -> p (t g)"), EXP)
            CES = sm.tile([128, TC], F32, tag="CES")
            nc.vector.tensor_reduce(CES[:], ES[:],
                                    mybir.AxisListType.X, ADD)
            nc.scalar.activation(CES[:], CES[:], LN_EXP)
            DD = sm.tile([128, TC], F32, tag="DD")
            nc.vector.tensor_sub(DD[:], SLB[:, :, 1], SLB[:, :, 0])
            nc.vector.tensor_tensor(DD[:], DD[:], STC[:], MULT)
            nc.vector.tensor_add(DD[:], DD[:], SLB[:, :, 0])
            nc.vector.tensor_sub(CES[:], CES[:], DD[:])

            R4 = sm.tile([128, 4], F32, tag="R4")
            W1 = sm.tile([128, TC], F32, tag="W1")
            nc.vector.tensor_tensor(W1[:], CES[:], MSC[:], MULT)
            W2 = sm.tile([128, TC], F32, tag="W2")
            nc.vector.tensor_tensor(W2[:], CEV[:], MVC[:], MULT)
            for i, srcbuf in enumerate([W1, MSC, W2, MVC]):
                rtc = tr.tile([128, 1], F32, tag="rtc")
                nc.vector.tensor_reduce(rtc[:], srcbuf[:],
                                        mybir.AxisListType.X, ADD)
                nc.vector.tensor_copy(R4[:, i:i + 1], rtc[:])

            nc.sync.dma_start(d_out[:], R4[:])

    nc.compile()
    return nc


def prepare_inputs(inputs):
    """Host-side: fold LN into weights, apply d2b permutation, fp8 packing,
    shard."""
    split = np.asarray(inputs["split"]).astype(np.int64)
    zq = np.asarray(inputs["zq"], dtype=np.float32)
    targets_vq = np.asarray(inputs["targets_vq"]).astype(np.int64)
    category = np.asarray(inputs["category"]).astype(np.int64)
    batch_id = np.asarray(inputs["batch_id"]).astype(np.int64)
    mask = np.asarray(inputs["mask"]).astype(bool)
    d2b = np.asarray(inputs["d2b"]).astype(np.int64)
    g = lambda k: np.asarray(inputs[k], dtype=np.float32)
    split_emb, class_emb = g("split_emb"), g("class_emb")
    vq_proj_w, vq_proj_b = g("vq_proj_w"), g("vq_proj_b")
    ln1_s, ln1_b = g("ln1_s"), g("ln1_b")
    qkv_w, qkv_b = g("qkv_w"), g("qkv_b")
    attn_w, attn_b = g("attn_w"), g("attn_b")
    ln2_s, ln2_b = g("ln2_s"), g("ln2_b")
    fc1_w, fc1_b = g("fc1_w"), g("fc1_b")
    fc2_w, fc2_b = g("fc2_w"), g("fc2_b")
    lnx_s, lnx_b = g("lnx_s"), g("lnx_b")
    split_w, split_b = g("split_w"), g("split_b")
    vq_w, vq_b = g("vq_w"), g("vq_b")

    # LN folds
    qkv_w_eff = ln1_s[:, :, None] * qkv_w                       # [L,C,3C]
    qkv_b_eff = np.einsum("lc,lcn->ln", ln1_b, qkv_w) + qkv_b   # [L,3C]
    fc1_w_eff = ln2_s[:, :, None] * fc1_w
    fc1_b_eff = np.einsum("lc,lcn->ln", ln2_b, fc1_w) + fc1_b
    vq_w_eff = lnx_s[:, None] * vq_w
    vq_b_eff = lnx_b @ vq_w + vq_b
    spl_w_eff = lnx_s[:, None] * split_w
    spl_b_eff = lnx_b @ split_w + split_b

    # natural col order [Q|K] fp8 x2S; V separate bf16 cc-major xS
    def pack_qkv(w):        # [C, 3C] -> [128, 2, 2C]
        qw, kw = w[:, 0:C], w[:, C:2 * C]
        cols = np.concatenate([qw * (2 * S), kw * (2 * S)], axis=1)
        return cols.reshape(128, 2, 2 * C)

    wqkv2 = np.stack([pack_qkv(qkv_w_eff[l]) for l in range(L)])
    wattn2 = (attn_w * S).reshape(L, 2, 128, C).transpose(0, 2, 1, 3)
    wfc12 = (fc1_w_eff * S).reshape(L, 128, 2, HID)
    wfc22 = (fc2_w * S).reshape(L, 8, 128, C).transpose(0, 2, 1, 3)
    wvq2 = (vq_w_eff * S).reshape(2, 128, VQ_G * VQ_SIZE).transpose(1, 0, 2)
    wspl2 = (spl_w_eff * S).reshape(2, 128, 2).transpose(1, 0, 2)
    wvv2 = (qkv_w_eff[:, :, 2 * C:3 * C] * S).reshape(L, 2, 128, C).transpose(0, 2, 1, 3)

    # biases (normally all zero)
    bqk2 = np.zeros((L, 128, 4), np.float32)
    for l in range(L):
        for g in range(4):
            qk, gg = divmod(g, 2)
            bqk2[l, :, g] = qkv_b_eff[l, qk * C + gg * 128:
                                      qk * C + gg * 128 + 128] * 2
    bv2 = np.tile(qkv_b_eff[:, 2 * C:3 * C], (1, 2))
    battn2 = np.tile(attn_b, (1, 2))
    bfc12 = fc1_b_eff.reshape(L, 8, 128).transpose(0, 2, 1)
    bfc22 = np.tile(fc2_b, (1, 2))
    bspl2 = np.tile(spl_b_eff, 16)

    # token embedding pieces, depth order
    cond_rows = class_emb[category[batch_id]]                   # [N,C]
    base_depth = np.empty((N, C), np.float32)
    base_depth[:N_SPLIT] = split_emb[split]
    base_depth[N_SPLIT:] = vq_proj_b[None, :]
    base_depth[mask] = cond_rows[mask]
    zq_depth = np.zeros((N, DH), np.float32)
    zq_depth[N_SPLIT:] = zq
    zq_depth[mask] = 0.0

    ms_depth = np.zeros(N, np.float32)
    ms_depth[:N_SPLIT] = mask[:N_SPLIT]
    mv_depth = np.zeros(N, np.float32)
    mv_depth[N_SPLIT:] = mask[N_SPLIT:]
    st_depth = np.zeros(N, np.float32)
    st_depth[:N_SPLIT] = split
    wsel_depth = np.zeros((N, C), np.float32)
    cols = targets_vq + np.arange(VQ_G)[None, :] * VQ_SIZE      # [N_VQ,4]
    wsel_depth[N_SPLIT:] = vq_w_eff.T[cols].sum(axis=1)         # [N_VQ,C]
    bsel_depth = np.zeros(N, np.float32)
    bsel_depth[N_SPLIT:] = vq_b_eff[cols].sum(axis=1)

    # window order + positional embedding
    pe = _sin_pos_emb(N, C)
    emb_w = base_depth[d2b] + pe
    zq_w = zq_depth[d2b]
    ms_w, mv_w, st_w = ms_depth[d2b], mv_depth[d2b], st_depth[d2b]
    wsel_w, bsel_w = wsel_depth[d2b] * S, bsel_depth[d2b]

    flags = {
        "bqkv": bool(np.any(qkv_b_eff[:, :2 * C])),
        "bqkv_v": bool(np.any(qkv_b_eff[:, 2 * C:])),
        "battn": bool(np.any(attn_b)),
        "bfc1": bool(np.any(fc1_b_eff)),
        "bfc2": bool(np.any(fc2_b)),
        "bspl": bool(np.any(spl_b_eff)),
        "bsel": bool(np.any(bsel_w)),
        "ebq": bool(np.any(vq_b_eff)),
    }

    shared = {
        "vqpw": vq_proj_w.astype(BF),
        "wqkv": wqkv2.astype(E4M3),
        "wattn": wattn2.astype(BF),
        "wfc1": wfc12.astype(E4M3),
        "wfc2": wfc22.astype(E4M3),
        "wvq": wvq2.astype(BF),
        "wspl": wspl2.astype(BF),
        "wvv": wvv2.astype(BF),
        "bqk": bqk2.astype(np.float32),
        "bv": bv2.astype(np.float32),
        "battn": battn2.astype(np.float32),
        "bfc1": bfc12.astype(np.float32),
        "bfc2": bfc22.astype(np.float32),
        "bspl": bspl2.astype(np.float32),
        "ebq": np.exp(vq_b_eff).astype(np.float32),
    }
    in_maps = []
    for c in range(NCORES):
        s = slice(c * T, (c + 1) * T)
        m = dict(shared)
        m["emb"] = np.ascontiguousarray(emb_w[s])
        m["zqt"] = np.ascontiguousarray(zq_w[s].T).astype(BF)
        m["wsel"] = np.ascontiguousarray(wsel_w[s])
        m["bsel"] = np.ascontiguousarray(bsel_w[s])
        m["msc"] = np.ascontiguousarray(ms_w[s])
        m["mvc"] = np.ascontiguousarray(mv_w[s])
        m["stc"] = np.ascontiguousarray(st_w[s])
        in_maps.append(m)
    return in_maps, flags


def kernel(**inputs) -> np.ndarray:
    in_maps, flags = prepare_inputs(inputs)
    key = tuple(sorted(flags.items()))
    if key not in _CACHE:
        _CACHE[key] = build_nc(flags)
    nc = _CACHE[key]
    res = run_bass_kernel_spmd(nc, in_maps, core_ids=list(range(NCORES)))
    parts = np.stack([res.results[c]["out"].sum(axis=0) for c in range(NCORES)])
    s = parts.sum(axis=0)
    split_loss = s[0] / max(s[1], 1.0)
    vq_loss = s[2] / max(s[3], 1.0)
    return np.stack([split_loss, vq_loss]).astype(np.float32)
